# revision 1
# baseline (speedup 1.0000x reference)
"""CQT (constant-Q transform) + amplitude_to_db kernel for Trainium2.

Full-input contract: kernel(x) takes x [32, 64000] f32 and returns
[32, 84, 126] f32, matching:

    frames = pad(x, n_fft//2)[:, t*HOP + n]          # [B, 126, 16384]
    cr/ci  = frames @ Kr.T / Ki.T                    # [B, 84, 126]
    mag    = sqrt(cr^2 + ci^2)
    out    = amplitude_to_db(mag, ref=max per item, amin=1e-5, top_db=80)

Sharding: pure data parallelism - 4 batch items per NeuronCore on 8 cores.

v2 design (hybrid precision):
  * The contraction over n_fft=16384 is split into 128-row chunks. Chunks are
    classified by weight mass: the 26+4 CENTRAL chunks (where the Hann
    windows of all bins peak) run in fp16; the 64 low-mass TAIL chunks (only
    bins 0-23 have support there) run as 32 fp8e4 DoubleRow pairs (c, c+4),
    processing two chunks per PE pass at 2x rate. Measured output rel err of
    this split is ~2.9e-3 (vs 1.6e-4 all-fp16, 1.8e-2 all-fp8).
  * Per-bin pow2 scales keep fp8/fp16 weights in range; matching scale
    corrections are applied per-partition in the ln-domain epilogue.
  * PSUM is pre-zeroed by warmup junk matmuls (which also hold the PE
    p-state ramp), so all real matmuls accumulate order-free (start=False).
  * Central fp16 matmuls are split into T-halves; the h0 epilogue
    (square/add/clamp/ln/reduce) overlaps the h1 matmuls.
  * dB epilogue in ln domain: out = max(DB_SCALE*(ln m2 - ln ref2), -80),
    ref from a free-dim max + GpSimd partition all-reduce; output fp16.
"""

import os
import numpy as np
import ml_dtypes
from contextlib import ExitStack

# "dbl" = fp8 DoubleRow pairs (2x PE rate); "flat" = plain fp8 per-chunk
# matmuls (bf16-rate fallback, for bisecting HW behavior)
TAIL_MODE = os.environ.get("CQT_TAIL_MODE", "dbl")
EPI_SAFE = int(os.environ.get("CQT_EPI_SAFE", "0"))

import concourse.bass as bass
import concourse.mybir as mybir
from concourse import bacc
from concourse import bass_isa
from concourse.ap import AP
from concourse.bass_utils import run_bass_kernel_spmd

# ---- problem constants (hardcoded; must match the reference) ----
SR = 22050
HOP = 512
N_BINS = 84
BPO = 12
FMIN = 32.70319566257483
AMIN = 1e-5
TOP_DB = 80.0
B = 32
N_SAMP = 64000
N_CORES = 8
NI = B // N_CORES            # items per core = 4
T = 1 + N_SAMP // HOP        # 126 frames
TH = T // 2                  # 63, T-half for epilogue overlap
DB_SCALE = 10.0 / np.log(10.0)  # 20*log10(mag) == DB_SCALE * ln(mag^2)
X8_SCALE = 32.0              # x prescale into fp8e4's sweet spot

P = 128
NLOW = 24                    # bins 0-23 live in the fp8 tail chunks
MB = 64                      # B-group stationary width (re 0:20, im 32:52)
NB_BINS = N_BINS - 64        # 20


def _build_cqt_kernels():
    """Same construction as the reference (nnAudio-style direct CQT bank)."""
    Q = 1.0 / (2.0 ** (1.0 / BPO) - 1.0)
    freqs = FMIN * 2.0 ** (np.arange(N_BINS) / BPO)
    lengths = np.ceil(Q * SR / freqs).astype(int)
    n_fft = int(2 ** np.ceil(np.log2(lengths.max())))
    K = np.zeros((N_BINS, n_fft), dtype=np.complex128)
    for k in range(N_BINS):
        L = int(lengths[k])
        t = np.arange(L) - (L - 1) / 2.0
        kern = np.hanning(L) * np.exp(2j * np.pi * freqs[k] * t / SR)
        kern /= np.abs(kern).sum()
        kern /= np.sqrt(L)
        s = (n_fft - L) // 2
        K[k, s:s + L] = kern
    return K.real.astype(np.float32), K.imag.astype(np.float32), n_fft


Kr, Ki, N_FFT = _build_cqt_kernels()
PAD = N_FFT // 2
FW = (N_SAMP + 2 * PAD) // P      # 628 free-dim width of column-major xp
QW = FW // 4                      # 157
assert (N_SAMP + 2 * PAD) % P == 0 and HOP == 4 * P

# per-bin pow2 scales: fp8 weight tiles use S8, fp16 tiles use S16 = 32*S8 so
# that psum accumulations (fp8 path has x pre-scaled by 32) are consistent.
_mx = np.maximum(np.abs(Kr).max(axis=1), np.abs(Ki).max(axis=1))
S8 = np.exp2(np.floor(np.log2(224.0 / _mx))).astype(np.float32)   # [84]
S16 = S8 * X8_SCALE
AMINV = ((AMIN * S16) ** 2).astype(np.float32)    # clamp operand (scaled dom)
INVS2 = (1.0 / (S16 * S16)).astype(np.float32)    # pow2 unscale, exact in f32

# ---- chunk classification ----
# A-group support (bins 0-63) is chunks [19, 109); oct2 (bins 24-35) support
# is [52, 76).  fp16 centrals: {51} u [52,76) u {108} (26 chunks, M=128).
# fp8 tails: [19,51) and [76,108) -> 2x32 chunks -> 32 DoubleRow pairs.
# B-group (bins 64-83): chunks [62, 66), fp16, M=64.
C16A = [51] + list(range(52, 76)) + [108]
C16B = [62, 63, 64, 65]


def _side_pairs(a, b):
    # pair (c, c+16): same phase, rhs slot step = 16 elements (the DoubleRow
    # ISA requires the Ko-dim step to be a multiple of 16)
    d = (b - a) // 2
    assert d == 16
    return [(c, c + d) for c in range(a, a + d)]


PAIRS = sorted(_side_pairs(19, 51) + _side_pairs(76, 108),
               key=lambda p: (p[0] % 4, p[0]))     # 32 pairs, phase-ordered
NPAIR = len(PAIRS)
M8 = 96          # fp8 stationary width: re 0:24, zeros, im 64:88, pad to 96
W8COLS = 2 * M8  # per pair tile (DoubleRow interleave)

# fp16 centrals in consumption order: phase{2,3} chunks first (their x16
# half is DMA'd first), B tiles first within each group so psB completes
# early and its squares hide under the remaining A matmuls
CENT = sorted([("A", c) for c in C16A] + [("B", c) for c in C16B],
              key=lambda kc: (0 if kc[1] % 4 >= 2 else 1,
                              0 if kc[0] == "B" else 1, kc[1]))
LAST_B = max(j for j, (k, _) in enumerate(CENT) if k == "B")
CENT_W = [P for k, _ in CENT]   # B tiles padded to 128 so FWL stays on
CENT_OFF = np.concatenate([[0], np.cumsum(CENT_W)]).astype(int)
W16COLS = int(CENT_OFF[-1])          # 26*128 + 4*64 = 3584

# w16 DMA slabs: tiles [0,10), [10,20), [20,30)
W16_SLAB = [0, 10, 20, 30]
# w8 DMA slabs (pair indices): [0,2), [2,12), [12,22), [22,32) -- slab 0 is
# tiny so it and x8 phase 1 both clear the scalar queue before they gate
W8_SLAB = [0, 2, 12, 22, 32]


def _pack_weights():
    KrT, KiT = Kr.T, Ki.T        # [N_FFT, 84]
    w16 = np.zeros((P, W16COLS), np.float32)
    for j, (kind, c) in enumerate(CENT):
        o = CENT_OFF[j]
        blk = slice(c * P, (c + 1) * P)
        if kind == "A":
            w16[:, o:o + 64] = KrT[blk, :64] * S16[:64]
            w16[:, o + 64:o + 128] = KiT[blk, :64] * S16[:64]
        else:
            w16[:, o:o + NB_BINS] = KrT[blk, 64:] * S16[64:]
            w16[:, o + 32:o + 32 + NB_BINS] = KiT[blk, 64:] * S16[64:]
            # cols o+52 .. o+128 stay zero (pad to full FWL width)
    w8 = np.zeros((P, NPAIR * W8COLS), np.float32)
    for j, (c0, c1) in enumerate(PAIRS):
        for s, c in enumerate((c0, c1)):
            o = j * W8COLS + s * M8
            blk = slice(c * P, (c + 1) * P)
            w8[:, o:o + NLOW] = KrT[blk, :NLOW] * S8[:NLOW]
            w8[:, o + 64:o + 64 + NLOW] = KiT[blk, :NLOW] * S8[:NLOW]
    return w16.astype(np.float16), w8.astype(ml_dtypes.float8_e4m3)


W16, W8 = _pack_weights()
CONSTS = np.stack([AMINV, INVS2], axis=1)     # [84, 2] f32


def build_program():
    nc = bacc.Bacc("TRN2", target_bir_lowering=False, debug=False,
                   enable_asserts=True)
    f32 = mybir.dt.float32
    f16 = mybir.dt.float16
    f8 = mybir.dt.float8e4

    x16_in = nc.dram_tensor("x16_in", [P, NI * FW], f16, kind="ExternalInput").ap()
    x8_in = nc.dram_tensor("x8_in", [P, NI * FW], f8, kind="ExternalInput").ap()
    w16_in = nc.dram_tensor("w16_in", [P, W16COLS], f16, kind="ExternalInput").ap()
    w8_in = nc.dram_tensor("w8_in", [P, NPAIR * W8COLS], f8,
                           kind="ExternalInput").ap()
    cst_in = nc.dram_tensor("cst_in", [N_BINS, 2], f32, kind="ExternalInput").ap()
    out = nc.dram_tensor("out", [N_BINS, NI * T], f16, kind="ExternalOutput").ap()

    xt16 = nc.alloc_sbuf_tensor("xt16", [P, NI * FW], f16).ap()
    xt8 = nc.alloc_sbuf_tensor("xt8", [P, NI * FW], f8).ap()
    wt16 = nc.alloc_sbuf_tensor("wt16", [P, W16COLS], f16).ap()
    wt8 = nc.alloc_sbuf_tensor("wt8", [P, NPAIR * W8COLS], f8).ap()
    cst = nc.alloc_sbuf_tensor("cst", [N_BINS, 2], f32).ap()
    junk = nc.alloc_sbuf_tensor("junk", [P, 512], f16).ap()
    tmp = nc.alloc_sbuf_tensor("tmp", [N_BINS, NI * T], f32).ap()
    m2 = nc.alloc_sbuf_tensor("m2", [N_BINS, NI * T], f32).ap()
    lnm = nc.alloc_sbuf_tensor("lnm", [N_BINS, NI * T], f32).ap()
    db = nc.alloc_sbuf_tensor("db", [N_BINS, NI * T], f16).ap()
    r2 = nc.alloc_sbuf_tensor("r2", [N_BINS, 2 * NI], f32).ap()
    r1 = nc.alloc_sbuf_tensor("r1", [N_BINS, NI], f32).ap()
    rall = nc.alloc_sbuf_tensor("rall", [N_BINS, NI], f32).ap()
    opv = nc.alloc_sbuf_tensor("opv", [N_BINS, NI], f32).ap()
    lnwarm = nc.alloc_sbuf_tensor("lnwarm", [1, 2], f32).ap()

    # one PSUM bank per T-half so the h0 epilogue can read its banks while
    # the PE is still accumulating the h1 banks (same-bank concurrent
    # read/accumulate wedges the hardware)
    psA2 = [nc.alloc_psum_tensor(f"psA{h}", [P, NI * TH], f32).ap()
            for h in range(2)]
    psB2 = [nc.alloc_psum_tensor(f"psB{h}", [P, NI * TH], f32).ap()
            for h in range(2)]
    psW = nc.alloc_psum_tensor("psW", [P, 512], f32).ap()

    s_x8 = [nc.alloc_semaphore(f"s_x8{r}") for r in range(4)]
    s_x16 = [nc.alloc_semaphore(f"s_x16{h}") for h in range(2)]
    s_w8 = [nc.alloc_semaphore(f"s_w8{i}") for i in range(4)]
    s_w16 = [nc.alloc_semaphore(f"s_w16{i}") for i in range(3)]
    s_cst = nc.alloc_semaphore("s_cst")
    s_mi = nc.alloc_semaphore("s_mi")
    s_pe = nc.alloc_semaphore("s_pe")     # 1 = h0 psums final, 2 = h1 final
    s_a = nc.alloc_semaphore("s_a")       # ACT epilogue steps
    s_v = nc.alloc_semaphore("s_v")       # DVE epilogue steps
    s_g2 = nc.alloc_semaphore("s_g2")     # gpsimd all-reduce done
    s_out = nc.alloc_semaphore("s_out")
    s_out2 = nc.alloc_semaphore("s_out2")

    # x SBUF layout is (r, q, i): col = r*628 + q*4 + i, so (t, i) merges
    # into one contiguous 252-wide moving dim per T-half and psum columns
    # are t-major/item-minor.
    HC = NI * TH                                  # 252 columns per T-half

    Ln = mybir.ActivationFunctionType.Ln
    Square = mybir.ActivationFunctionType.Square
    Amax = mybir.AluOpType.max
    Asub = mybir.AluOpType.subtract
    Aadd = mybir.AluOpType.add
    Amult = mybir.AluOpType.mult

    def hslice(ap_pit, h):
        return ap_pit[:, h * HC:(h + 1) * HC]

    def cent_rhs(c, h):
        r, q0 = c % 4, c // 4
        o = r * FW + (q0 + h * TH) * NI
        return xt16[:, o:o + HC]

    def pair_rhs(c, h):
        """DoubleRow rhs [128][2 slots][252] for pair (c, c+4), T-half h."""
        r, q0 = c % 4, c // 4
        o = r * FW + (q0 + h * TH) * NI
        base = xt8[:, o:o + HC]
        ap_l = [list(d) for d in base.ap]
        return AP(base.tensor, base.offset,
                  [ap_l[0], [16, 2], [1, HC]])

    def w8_sem(j):
        for i in range(4):
            if j < W8_SLAB[i + 1]:
                return s_w8[i]
        raise IndexError(j)

    def w16_sem(j):
        for i in range(3):
            if j < W16_SLAB[i + 1]:
                return s_w16[i]
        raise IndexError(j)

    if TAIL_MODE == "mmonly":
        with nc.Block() as block:
            @block.sync
            def _(sync):
                sync.dma_start(xt8[:, 0:NI * QW],
                               x8_in[:, 0:NI * QW]).then_inc(s_x8[0], 16)
                o0, o1 = W8_SLAB[0] * W8COLS, W8_SLAB[1] * W8COLS
                sync.dma_start(wt8[:, o0:o1], w8_in[:, o0:o1]).then_inc(s_w8[0], 16)
                sync.dma_start(xt16[:, 0:2 * NI * QW],
                               x16_in[:, 0:2 * NI * QW]).then_inc(s_x16[0], 16)
                t0, t1 = CENT_OFF[W16_SLAB[1]], CENT_OFF[W16_SLAB[2]]
                sync.dma_start(wt16[:, t0:t1],
                               w16_in[:, t0:t1]).then_inc(s_w16[1], 16)
                sync.wait_ge(s_a, {0: 1, 1: 1, 2: 11, 3: 9, 4: 9}[EPI_SAFE])
                sync.dma_start(out[:, 0:HC], db[:, 0:HC]).then_inc(s_out, 16)
                sync.wait_ge(s_out, 16)

            @block.scalar
            def _(scalar):
                scalar.dma_start(xt8[:, NI * QW:2 * NI * QW],
                                 x8_in[:, NI * QW:2 * NI * QW]).then_inc(s_x8[1], 16)
                o0, o1 = W8_SLAB[1] * W8COLS, W8_SLAB[2] * W8COLS
                scalar.dma_start(wt8[:, o0:o1], w8_in[:, o0:o1]).then_inc(s_w8[1], 16)
                t0, t1 = CENT_OFF[W16_SLAB[0]], CENT_OFF[W16_SLAB[1]]
                scalar.dma_start(wt16[:, t0:t1],
                                 w16_in[:, t0:t1]).then_inc(s_w16[0], 16)
                scalar.dma_start(xt8[:, 3 * NI * QW:4 * NI * QW],
                                 x8_in[:, 3 * NI * QW:4 * NI * QW]
                                 ).then_inc(s_x8[3], 16)
                if EPI_SAFE >= 2:
                    scalar.activation(lnwarm[:, 0:1],
                                      nc.const_aps.tensor(1.0, (1, 1)), Ln)
                    scalar.activation(lnwarm[:, 1:2],
                                      nc.const_aps.tensor(1.0, (1, 1)), Square)
                    for h in range(2):
                        scalar.wait_ge(s_pe, h + 1)
                        scalar.activation(hslice(m2[0:64], h),
                                          psA2[h][0:64],
                                          Square).then_inc(s_a)
                        scalar.activation(hslice(tmp[0:64], h),
                                          psA2[h][64:128],
                                          Square).then_inc(s_a)
                        scalar.activation(hslice(m2[64:84], h),
                                          psB2[h][0:20],
                                          Square).then_inc(s_a)
                        scalar.activation(hslice(tmp[64:84], h),
                                          psB2[h][32:52],
                                          Square).then_inc(s_a)
                        if EPI_SAFE == 2:
                            scalar.wait_ge(s_v, h + 1)
                            scalar.activation(hslice(lnm, h), hslice(m2, h),
                                              Ln).then_inc(s_a)
                    if EPI_SAFE == 4:
                        scalar.wait_ge(s_v, 2)
                    scalar.activation(db[:], m2[0:N_BINS],
                                      mybir.ActivationFunctionType.Copy
                                      ).then_inc(s_a)
                else:
                    scalar.wait_ge(s_pe, 2)
                    scalar.activation(db[:, 0:HC], psA2[0][0:N_BINS],
                                      mybir.ActivationFunctionType.Copy
                                      ).then_inc(s_a)

            @block.gpsimd
            def _(gpsimd):
                gpsimd.memset(junk[:], 0.0).then_inc(s_mi, 1)
                o0, o1 = W8_SLAB[2] * W8COLS, W8_SLAB[3] * W8COLS
                gpsimd.dma_start(wt8[:, o0:o1], w8_in[:, o0:o1]).then_inc(s_w8[2], 16)
                gpsimd.dma_start(xt8[:, 2 * NI * QW:3 * NI * QW],
                                 x8_in[:, 2 * NI * QW:3 * NI * QW]
                                 ).then_inc(s_x8[2], 16)
                o0, o1 = W8_SLAB[3] * W8COLS, W8_SLAB[4] * W8COLS
                gpsimd.dma_start(wt8[:, o0:o1], w8_in[:, o0:o1]).then_inc(s_w8[3], 16)
                gpsimd.dma_start(cst[:], cst_in).then_inc(s_cst, 16)
                t0, t1 = CENT_OFF[W16_SLAB[2]], CENT_OFF[W16_SLAB[3]]
                gpsimd.dma_start(wt16[:, t0:t1],
                                 w16_in[:, t0:t1]).then_inc(s_w16[2], 16)
                gpsimd.dma_start(xt16[:, 2 * NI * QW:4 * NI * QW],
                                 x16_in[:, 2 * NI * QW:4 * NI * QW]
                                 ).then_inc(s_x16[1], 16)
                gpsimd.wait_ge(s_a, {0: 1, 1: 1, 2: 11, 3: 9, 4: 9}[EPI_SAFE])
                gpsimd.dma_start(out[:, HC:], db[:, HC:]).then_inc(s_out2, 16)
                gpsimd.wait_ge(s_out2, 16)

            @block.vector
            def _(vector):
                vector.wait_ge(s_cst, 16)
                if EPI_SAFE in (2, 4):
                    for h in range(2):
                        vector.wait_ge(s_a, (5 if EPI_SAFE == 2 else 4) * h + 4)
                        vector.tensor_tensor(hslice(m2, h), hslice(m2, h),
                                             hslice(tmp, h), Aadd)
                        vector.drain()
                        vector.tensor_scalar_max(hslice(m2, h), hslice(m2, h),
                                                 cst[:, 1:2])
                        vector.drain().then_inc(s_v, 1)
                        if EPI_SAFE == 2:
                            vector.wait_ge(s_a, 5 * h + 5)
                            vector.tensor_reduce(
                                r2[:, h * NI:(h + 1) * NI],
                                lnm.rearrange("p (t i) -> p i t",
                                              i=NI)[:, :, h * TH:(h + 1) * TH],
                                axis=mybir.AxisListType.X, op=Amax)
                    vector.drain()

            @block.tensor
            def _(tensor):
                tensor.wait_ge(s_mi, 1)
                for _ in range(6):
                    tensor.matmul(psW[:], lhsT=junk[:, :P], rhs=junk[:, :512],
                                  start=True, stop=True)
                for h in range(2):
                    tensor.matmul(psA2[h][:], lhsT=junk[:, :P],
                                  rhs=junk[:, :NI * TH], start=True,
                                  stop=True, skip_group_check=True)
                    tensor.matmul(psB2[h][:], lhsT=junk[:, :P],
                                  rhs=junk[:, :NI * TH], start=True,
                                  stop=True, skip_group_check=True)
                waited = set()

                def need(sem):
                    if id(sem) not in waited:
                        tensor.wait_ge(sem, 16)
                        waited.add(id(sem))

                for h in range(2):
                    for j, (kind, c) in enumerate(CENT):
                        need(s_x16[0 if c % 4 < 2 else 1])
                        need(w16_sem(j))
                        rhs = cent_rhs(c, h)
                        o = CENT_OFF[j]
                        if kind == "A":
                            tensor.matmul(psA2[h][:],
                                          lhsT=wt16[:, o:o + P], rhs=rhs,
                                          start=False,
                                          stop=(h == 1 and j == len(CENT) - 1),
                                          skip_group_check=True)
                        else:
                            tensor.matmul(psB2[h][:],
                                          lhsT=wt16[:, o:o + P], rhs=rhs,
                                          start=False, stop=False,
                                          skip_group_check=True)
                    tensor.drain().then_inc(s_pe, 1)

        nc.compile()
        return nc

    with nc.Block() as block:

        @block.sync
        def _(sync):
            o0, o1 = W8_SLAB[1] * W8COLS, W8_SLAB[2] * W8COLS
            sync.dma_start(wt8[:, o0:o1], w8_in[:, o0:o1]).then_inc(s_w8[1], 16)
            sync.dma_start(xt8[:, 3 * NI * QW:4 * NI * QW],
                           x8_in[:, 3 * NI * QW:4 * NI * QW]).then_inc(s_x8[3], 16)
            sync.dma_start(xt16[:, 2 * NI * QW:4 * NI * QW],
                           x16_in[:, 2 * NI * QW:4 * NI * QW]).then_inc(s_x16[1], 16)
            sync.dma_start(xt16[:, 0:2 * NI * QW],
                           x16_in[:, 0:2 * NI * QW]).then_inc(s_x16[0], 16)
            t0, t1 = CENT_OFF[W16_SLAB[1]], CENT_OFF[W16_SLAB[2]]
            sync.dma_start(wt16[:, t0:t1], w16_in[:, t0:t1]).then_inc(s_w16[1], 16)
            sync.wait_ge(s_v, 4)
            sync.dma_start(out[:, 0:2 * T], db[:, 0:2 * T]).then_inc(s_out, 16)
            sync.wait_ge(s_out, 16)

        @block.scalar
        def _(scalar):
            o0, o1 = W8_SLAB[0] * W8COLS, W8_SLAB[1] * W8COLS
            scalar.dma_start(wt8[:, o0:o1], w8_in[:, o0:o1]).then_inc(s_w8[0], 16)
            scalar.dma_start(xt8[:, NI * QW:2 * NI * QW],
                             x8_in[:, NI * QW:2 * NI * QW]).then_inc(s_x8[1], 16)
            o0, o1 = W8_SLAB[3] * W8COLS, W8_SLAB[4] * W8COLS
            scalar.dma_start(wt8[:, o0:o1], w8_in[:, o0:o1]).then_inc(s_w8[3], 16)
            t0, t1 = CENT_OFF[W16_SLAB[0]], CENT_OFF[W16_SLAB[1]]
            scalar.dma_start(wt16[:, t0:t1], w16_in[:, t0:t1]).then_inc(s_w16[0], 16)
            # preload BOTH act table slots (Ln + Square) while DMAs fly
            scalar.activation(lnwarm[:, 0:1], nc.const_aps.tensor(1.0, (1, 1)), Ln)
            scalar.activation(lnwarm[:, 1:2], nc.const_aps.tensor(1.0, (1, 1)),
                              Square)
            for h in range(2):
                # cross-partition ACT squares: re^2 -> m2, im^2 -> tmp, both
                # landing at the bin's partition so the DVE add is aligned;
                # B squares run while the PE finishes the A chunks of this half
                scalar.wait_ge(s_pe, 2 * h + 1)
                scalar.activation(hslice(m2[64:84], h), psB2[h][0:20],
                                  Square).then_inc(s_a)
                scalar.activation(hslice(tmp[64:84], h), psB2[h][32:52],
                                  Square).then_inc(s_a)
                scalar.wait_ge(s_pe, 2 * h + 2)
                scalar.activation(hslice(m2[0:64], h), psA2[h][0:64],
                                  Square).then_inc(s_a)
                scalar.activation(hslice(tmp[0:64], h), psA2[h][64:128],
                                  Square).then_inc(s_a)
                scalar.wait_ge(s_v, h + 1)
                # ln pass relayouts (t,i) -> (i,t) so the per-item final
                # tensor_scalar reads/writes contiguously
                scalar.activation(
                    lnm.rearrange("p (i t) -> p t i",
                                  i=NI)[:, h * TH:(h + 1) * TH],
                    m2.rearrange("p (t i) -> p t i",
                                 i=NI)[:, h * TH:(h + 1) * TH],
                    Ln).then_inc(s_a)
            scalar.wait_ge(s_g2, 1)
            scalar.activation(opv[:], rall[:], Ln).then_inc(s_a)   # s_a == 11


        @block.gpsimd
        def _(gpsimd):
            gpsimd.memset(junk[:], 0.0).then_inc(s_mi, 1)
            gpsimd.dma_start(xt8[:, 0:NI * QW],
                             x8_in[:, 0:NI * QW]).then_inc(s_x8[0], 16)
            gpsimd.dma_start(xt8[:, 2 * NI * QW:3 * NI * QW],
                             x8_in[:, 2 * NI * QW:3 * NI * QW]).then_inc(s_x8[2], 16)
            o0, o1 = W8_SLAB[2] * W8COLS, W8_SLAB[3] * W8COLS
            gpsimd.dma_start(wt8[:, o0:o1], w8_in[:, o0:o1]).then_inc(s_w8[2], 16)
            gpsimd.dma_start(cst[:], cst_in).then_inc(s_cst, 16)
            t0, t1 = CENT_OFF[W16_SLAB[2]], CENT_OFF[W16_SLAB[3]]
            gpsimd.dma_start(wt16[:, t0:t1], w16_in[:, t0:t1]).then_inc(s_w16[2], 16)
            gpsimd.wait_ge(s_v, 3)
            gpsimd.partition_all_reduce(rall[:], r1[:], channels=N_BINS,
                                        reduce_op=bass_isa.ReduceOp.max
                                        ).then_inc(s_g2, 1)
            gpsimd.wait_ge(s_v, 5)
            gpsimd.dma_start(out[:, 2 * T:], db[:, 2 * T:]).then_inc(s_out2, 16)
            gpsimd.wait_ge(s_out2, 16)


        @block.vector
        def _(vector):
            vector.wait_ge(s_cst, 16)
            for h in range(2):
                vector.wait_ge(s_a, 5 * h + 4)
                vector.tensor_tensor(hslice(m2, h), hslice(m2, h),
                                     hslice(tmp, h), Aadd)
                vector.drain()
                vector.tensor_scalar(hslice(m2, h), hslice(m2, h),
                                     cst[:, 0:1], cst[:, 1:2], Amax, Amult)
                vector.drain().then_inc(s_v, 1)
                # r2 half: per-item free max of the clamped m2 half -- max
                # commutes with the monotone ln, so this runs in parallel
                # with the ACT Ln pass; ln(ref2) is recovered on [84,4] after
                # the all-reduce.
                vector.tensor_reduce(
                    r2[:, h * NI:(h + 1) * NI],
                    m2.rearrange("p (t i) -> p i t", i=NI)[:, :,
                                                           h * TH:(h + 1) * TH],
                    axis=mybir.AxisListType.X, op=Amax)
            vector.drain()
            vector.tensor_tensor(r1[:], r2[:, 0:NI], r2[:, NI:2 * NI], Amax)
            vector.drain().then_inc(s_v, 1)       # s_v == 3 -> gpsimd allreduce
            vector.wait_ge(s_a, 11)
            # db stores (lnm - lnr) clamped at -TOP_DB/DB_SCALE; the host
            # multiplies by DB_SCALE (exact linear op) after gathering.
            # items 0,1 first: their DMA rides the slower issuer (sync) and
            # overlaps the remaining tensor_scalar work for items 2,3
            for i in range(NI):
                vector.tensor_scalar(db[:, i * T:(i + 1) * T],
                                     lnm[:, i * T:(i + 1) * T],
                                     opv[:, i:i + 1],
                                     -float(TOP_DB / DB_SCALE),
                                     Asub, Amax)
                if i == 1:
                    vector.drain().then_inc(s_v, 1)   # s_v == 4 -> out0 DMA
            vector.drain().then_inc(s_v, 1)           # s_v == 5 -> out1 DMA

        @block.tensor
        def _(tensor):
            tensor.wait_ge(s_mi, 1)
            # warmup: hold the PE p-state ramp until real data lands; the
            # trailing junk matmuls pre-zero psA/psB so every real matmul
            # can accumulate order-free with start=False.
            for _ in range(7):
                tensor.matmul(psW[:], lhsT=junk[:, :P], rhs=junk[:, :512],
                              start=True, stop=True)
            for h in range(2):
                tensor.matmul(psA2[h][:], lhsT=junk[:, :P],
                              rhs=junk[:, :NI * TH], start=True,
                              stop=True, skip_group_check=True)
                tensor.matmul(psB2[h][:], lhsT=junk[:, :P],
                              rhs=junk[:, :NI * TH], start=True,
                              stop=True, skip_group_check=True)

            waited = set()

            def need(sem):
                if id(sem) not in waited:
                    tensor.wait_ge(sem, 16)
                    waited.add(id(sem))

            # fp8 DoubleRow tail pairs, phase-ordered (both T-halves)
            for j, (c0, c1) in (
                    [] if TAIL_MODE == "none" else list(enumerate(PAIRS))):
                need(s_x8[c0 % 4])
                need(w8_sem(j))
                if TAIL_MODE == "dbl":
                    wtile = wt8[:, j * W8COLS:(j + 1) * W8COLS].rearrange(
                        "p (two m) -> p two m", two=2)
                    for h in range(2):
                        tensor.matmul(psA2[h][0:M8, :],
                                      lhsT=wtile, rhs=pair_rhs(c0, h),
                                      start=False, stop=False,
                                      perf_mode=mybir.MatmulPerfMode.DoubleRow,
                                      skip_group_check=True)
                else:
                    for s, c in enumerate((c0, c1)):
                        o = j * W8COLS + s * M8
                        r, q0 = c % 4, c // 4
                        xo = r * FW + q0 * NI
                        for h in range(2):
                            tensor.matmul(psA2[h][0:M8, :],
                                          lhsT=wt8[:, o:o + M8],
                                          rhs=xt8[:, xo + h * HC:
                                                  xo + (h + 1) * HC],
                                          start=False, stop=False,
                                          skip_group_check=True)

            # fp16 centrals, T-half split for epilogue overlap; psB finishes
            # early (B tiles lead) so its squares hide under the A matmuls
            for h in range(2):
                for j, (kind, c) in enumerate(CENT):
                    need(s_x16[0 if c % 4 < 2 else 1])
                    need(w16_sem(j))
                    rhs = cent_rhs(c, h)
                    o = CENT_OFF[j]
                    if kind == "A":
                        tensor.matmul(psA2[h][:],
                                      lhsT=wt16[:, o:o + P], rhs=rhs,
                                      start=False,
                                      stop=(h == 1 and j == len(CENT) - 1),
                                      skip_group_check=True)
                    else:
                        tensor.matmul(psB2[h][:],
                                      lhsT=wt16[:, o:o + P], rhs=rhs,
                                      start=False, stop=False,
                                      skip_group_check=True)
                    if j == LAST_B:
                        tensor.drain().then_inc(s_pe, 1)   # psB[h] final
                tensor.drain().then_inc(s_pe, 1)           # psA[h] final

    nc.compile()
    return nc


def pack_x(x):
    """x [B, 64000] f32 -> per-core phase-major packs (f16, f8)."""
    xp = np.pad(np.asarray(x, dtype=np.float32), ((0, 0), (PAD, PAD)))
    # x_cm[b, p, r, q] = xp[b, (4q+r)*128+p]; chunk c=4q0+r streams in t
    x_cm = xp.reshape(B, QW, 4, P).transpose(0, 3, 2, 1)   # [B,128,4,157]
    packs = []
    for core in range(N_CORES):
        blk = x_cm[core * NI:(core + 1) * NI]              # [NI,128,4,157]
        lay = np.ascontiguousarray(
            blk.transpose(1, 2, 3, 0).reshape(P, NI * FW))  # [p, (r q i)]
        p16 = lay.astype(np.float16)
        p8 = np.clip(lay * X8_SCALE, -240.0, 240.0).astype(ml_dtypes.float8_e4m3)
        packs.append((p16, p8))
    return packs


_PROGRAM = None


def _get_program():
    global _PROGRAM
    if _PROGRAM is None:
        _PROGRAM = build_program()
    return _PROGRAM


def run(x, **spmd_kwargs):
    """Run on 8 NeuronCores; returns (output [32,84,126] f32, results)."""
    nc = _get_program()
    packs = pack_x(x)
    in_maps = [{"x16_in": packs[i][0], "x8_in": packs[i][1],
                "w16_in": W16, "w8_in": W8, "cst_in": CONSTS}
               for i in range(N_CORES)]
    res = run_bass_kernel_spmd(nc, in_maps, core_ids=list(range(N_CORES)),
                               **spmd_kwargs)
    outs = []
    for i in range(N_CORES):
        o = res.results[i]["out"].astype(np.float32)        # [84, (i t)]
        outs.append(o.reshape(N_BINS, NI, T).transpose(1, 0, 2))
    full = np.concatenate(outs, axis=0) * np.float32(DB_SCALE)
    return np.ascontiguousarray(full.astype(np.float32)), res


def kernel(x):
    return run(x)[0]

